# revision 1
# baseline (speedup 1.0000x reference)
"""Trainium2 Bass kernel for nn_AllAmplitude (helicity-amplitude intensity).

Math: the reference contracts two spin-1 Wigner-D matrices per (resonance,
event) with a Breit-Wigner weight and sums |amp|^2 over external helicities.
Because D1 @ D2 = D^1(U1 U2) for the SU(2) elements U1, U2 of the two
rotations, and sum_{a,dlt} mult_dlt M_r conj(M_r') = 2 tr D(V) + D(V)[0,0]
with V = U_r'^dag U_r, the whole intensity collapses to

  I = 7 sum_r |w_r|^2
    + sum_{r<r'} 2 Re(w_r conj(w_r')) (10 Re(av)^2 + 2 Im(av)^2 - 3)

with av = conj(a_r') a_r + b_r' conj(b_r), (a, b) the Cayley-Klein parameters
of the composed rotation, and w_r the complex Breit-Wigner weight.  Per event
this is ~200 flops instead of the reference's ~3000.

Sharding: pure data parallelism. The event axis N=262144 splits across the 8
NeuronCores (32768 events each, laid out [128 partitions x 256 events] with
the R=4 resonance slices side by side in the free axis).

Implementation (raw bass Block, manual semaphores; no Tile scheduler):
 - sin/cos of the four composite half-angles via fractional-turn range
   reduction (round-to-nearest through the 1.5*2^23 float trick), one fused
   custom-DVE op per angle/offset, feeding the ScalarE Sin spline;
 - chi and |w|^2 via a fused  a^2*s0 + b^2*s1 + s2  custom-DVE op;
 - bounded amplitude algebra in fp16 (2x DVE rate), Breit-Wigner in fp32,
   |w| pair products in bf16 (range);
 - ScalarE carries all transcendentals plus the 1-input affine ops;
 - per-resonance DMA-slice semaphores let the DVE start while inputs land.
"""

import numpy as np
from contextlib import ExitStack

import concourse.bass as bass
import concourse.tile as tile
from concourse import bacc, mybir
from concourse.bass_utils import run_bass_kernel_spmd

F32 = mybir.dt.float32
BF16 = mybir.dt.bfloat16
FP16 = mybir.dt.float16
ALU = mybir.AluOpType
ACTF = mybir.ActivationFunctionType

R = 4
N_TOTAL = 262144
N_CORES = 8
N_CORE = N_TOTAL // N_CORES     # 32768 events per core
P = 128                         # SBUF partitions
E = N_CORE // P                 # 256 events per partition per resonance
W = R * E                       # 1024 free-dim of a full working tile

MAGIC = float(np.float32(1.5 * 2.0**23))   # round-to-nearest-int bias trick
INV4PI = float(np.float32(1.0 / (4.0 * np.pi)))
TWOPI = float(np.float32(2.0 * np.pi))
HALFPI = float(np.float32(np.pi / 2.0))


INPUT_NAMES = ("alpha1", "beta1", "gamma1", "alpha2", "beta2", "gamma2", "m")


# ---------------------------------------------------------------------------
# custom fused DVE ops
# ---------------------------------------------------------------------------

def _register_custom_ops():
    import concourse.dve_ops as dve_ops
    from concourse.dve_spec import Spec, Src0, Src1, C0, C1, C2, sq, lower, _has_src1
    from concourse.dve_uop import DveOpSpec
    from concourse.dve_ops import DveOp

    if any(op.name == "ANT_RANGE_RED_ADD" for op in dve_ops.OPS):
        return {op.name: op for op in dve_ops.OPS}

    def make_op(name, spec):
        shas = {}
        for ver in ("v3", "v4"):
            uops = lower(spec, ver=ver)
            shas[ver] = DveOpSpec(name=name, opcode=31, uops=uops,
                                  rd1_en=_has_src1(spec)).sha(ver)
        return DveOp(name, spec, subdim=False, uops_sha=shas)

    def _rr_ref(sgn):
        def ref(in0, in1, s0, s1, imm2):
            t = ((in0 + sgn * in1) * s0 + s1).astype(np.float32)
            r = ((t + imm2).astype(np.float32) - imm2).astype(np.float32)
            return (t - r).astype(np.float32)
        return ref

    u = (Src0 + Src1) * C0 + C1
    rr_add = make_op("ANT_RANGE_RED_ADD",
                     Spec(body=u - ((u + C2) - C2), reference=_rr_ref(1.0)))
    u2 = (Src0 - Src1) * C0 + C1
    rr_sub = make_op("ANT_RANGE_RED_SUB",
                     Spec(body=u2 - ((u2 + C2) - C2), reference=_rr_ref(-1.0)))
    chi = make_op("ANT_CHI", Spec(
        body=sq(Src0) * C0 + sq(Src1) * C1 + C2,
        reference=lambda in0, in1, s0, s1, imm2:
            (in0 * in0 * s0 + in1 * in1 * s1 + imm2).astype(np.float32)))
    den = make_op("ANT_DEN", Spec(
        body=sq(C0 - Src0) + C1,
        reference=lambda in0, in1, s0, s1, imm2:
            ((s0 - in0) * (s0 - in0) + s1).astype(np.float32)))

    for op in (rr_add, rr_sub, chi, den):
        dve_ops.OPS.append(op)
        dve_ops._SUB_OPCODE_FOR_NAME[op.name] = (
            dve_ops._CUSTOM_DVE_ROW_BASE + len(dve_ops.OPS) - 1)
        dve_ops.CUSTOM_DVE_SPECS[op.name] = op.spec
    assert max(dve_ops._SUB_OPCODE_FOR_NAME.values()) < 0x20
    return {op.name: op for op in dve_ops.OPS}


def _rs(r):
    return slice(r * E, (r + 1) * E)


def build(m0, g0, coef_r, coef_i):
    OPS = _register_custom_ops()
    RR_ADD, RR_SUB, CHI, DEN = (OPS["ANT_RANGE_RED_ADD"], OPS["ANT_RANGE_RED_SUB"],
                                OPS["ANT_CHI"], OPS["ANT_DEN"])
    AT = FP16

    nc = bacc.Bacc("TRN2", target_bir_lowering=False, debug=False,
                   num_devices=N_CORES)
    ins = {k: nc.dram_tensor(k, (R, N_CORE), F32, kind="ExternalInput").ap()
           for k in INPUT_NAMES}
    out_ap = nc.dram_tensor("out", (N_CORE,), F32, kind="ExternalOutput").ap()

    f32 = np.float32
    m0 = m0.astype(np.float64); g0 = g0.astype(np.float64)
    cR = [float(f32(coef_r[r] * np.cos(coef_i[r]))) for r in range(R)]
    cI = [float(f32(coef_r[r] * np.sin(coef_i[r]))) for r in range(R)]
    m0sq = [float(f32(m0[r] * m0[r])) for r in range(R)]
    y = [float(f32(m0[r] * g0[r])) for r in range(R)]
    ysq = [float(f32(f32(y[r]) * f32(y[r]))) for r in range(R)]
    k1 = [float(f32(-f32(cI[r]) * f32(y[r]))) for r in range(R)]
    k2 = [float(f32(f32(cR[r]) * f32(y[r]))) for r in range(R)]

    # ---- static SBUF allocation ----
    alloc = []
    def sb(name, shape, dt=F32):
        t = nc.alloc_sbuf_tensor(name, list(shape), dt)
        alloc.append(t)
        return t.ap()

    tin = {k: sb(f"in_{k}", [P, W]) for k in INPUT_NAMES}
    pi2 = sb("pi2", [P, 1])
    ub = sb("ub", [P, W]); vb = sb("vb", [P, W])
    wb = sb("wb", [P, W]); zb = sb("zb", [P, W])
    fts = {n: sb(f"f_{n}", [P, W]) for n in
           ("As", "Ac", "Bs", "Bc", "Cs", "Cc", "Ds", "Dc")}
    sct = {n: sb(f"sc_{n}", [P, W], AT) for n in fts}
    cb1 = sb("cb1", [P, W], AT); sb1 = sb("sb1", [P, W], AT)
    cb2 = sb("cb2", [P, W], AT); sb2_ = sb("sb2", [P, W], AT)
    Ms = [sb(f"M{i}", [P, W], AT) for i in range(4)]
    pqs = {n: sb(f"pq_{n}", [P, W], AT) for n in fts}
    AB4 = sb("AB4", [P, 4 * W], AT)   # [are|aim|bre|bim]
    are = AB4[:, 0:W]; aim = AB4[:, W:2*W]
    bre = AB4[:, 2*W:3*W]; bim = AB4[:, 3*W:4*W]
    msq = sb("msq", [P, W]); den = sb("den", [P, W]); rcp = den
    wp1 = sb("wp1", [P, W]); wp2 = sb("wp2", [P, W])
    wre = sb("wre", [P, W]); wim = sb("wim", [P, W])
    WH = sb("WH", [P, 2 * W], BF16)   # [wreh|wimh]
    wreh = WH[:, 0:W]; wimh = WH[:, W:2*W]
    dall = sb("dall", [P, W]); dh = sb("dh", [P, 2 * E]); dg = sb("dg", [P, E])
    acc = sb("acc", [P, E])
    # pair scratch (reused across the 3 shift groups; DVE program order)
    NP3 = 3 * E
    PT = sb("PT", [P, 4 * NP3], AT)    # 4 packed products
    S1 = sb("S1", [P, 2 * NP3], AT)
    QT = sb("QT", [P, 4 * NP3], AT)    # [q1|q4|q2|q3]
    DT = sb("DT", [P, 2 * NP3], AT)
    NP6 = 6 * E
    avr = sb("avr", [P, NP6], AT); avi = sb("avi", [P, NP6], AT)
    chis = sb("chis", [P, NP6], BF16); gw = sb("gw", [P, NP6], BF16)
    GT6 = sb("GT6", [P, 2 * NP6], BF16)
    term = sb("term", [P, NP6], BF16)

    sem_ab = [nc.alloc_semaphore(f"s_ab{r}") for r in range(R)]
    sem_gz = [nc.alloc_semaphore(f"s_gz{r}") for r in range(R)]
    with (
        nc.semaphore("s_b1") as sem_b1,
        nc.semaphore("s_b2") as sem_b2,
        nc.semaphore("s_m") as sem_m,
        nc.semaphore("s_out") as sem_out,
        nc.semaphore("act_sem") as act_sem,
        nc.semaphore("vec_sem") as vec_sem,
        nc.Block() as block,
    ):
        # ------------- SYNC: DMAs (group semaphores: queue completions are
        # out-of-order across the 16 HWDGE queues) -------------
        @block.sync
        def _(sync):
            def dma(k, r, sem):
                sync.dma_start(
                    tin[k][:, _rs(r)],
                    ins[k][r].rearrange("(p e) -> p e", p=P, e=E),
                ).then_inc(sem, 16)

            for r in (0, 1):
                dma("alpha1", r, sem_ab[r]); dma("alpha2", r, sem_ab[r])
            for r in (0, 1):
                dma("gamma1", r, sem_gz[r]); dma("gamma2", r, sem_gz[r])
            for r in (2, 3):
                dma("alpha1", r, sem_ab[r]); dma("alpha2", r, sem_ab[r])
            for r in (2, 3):
                dma("gamma1", r, sem_gz[r]); dma("gamma2", r, sem_gz[r])
            for r in range(R):
                dma("beta1", r, sem_b1)
            for r in range(R):
                dma("beta2", r, sem_b2)
            for r in range(R):
                dma("m", r, sem_m)
            sync.wait_ge(vec_sem, 9)
            sync.dma_start(out_ap.rearrange("(p e) -> p e", p=P, e=E),
                           acc[:]).then_inc(sem_out, 16)
            sync.wait_ge(sem_out, 16)

        # ------------- SCALAR (ACT) -------------
        # 1 cb1, 2 sb1, 3 cb2, 4 sb2, 5 snA, 6 csA, 7 snB, 8 csB, 9 msq,
        # 10-13 wp1, 14-17 wp2, 18 snC, 19 csC, 20 snD, 21 csD,
        # 22 wreh, 23 wimh, 24 acc-init
        @block.scalar
        def _(scalar):
            scalar.wait_ge(vec_sem, 1)   # pi2 memset
            scalar.wait_ge(sem_b1, 64)
            scalar.activation(cb1[:], tin["beta1"][:], ACTF.Sin, scale=0.5,
                              bias=pi2[:]).then_inc(act_sem, 1)        # 1
            scalar.activation(sb1[:], tin["beta1"][:], ACTF.Sin,
                              scale=0.5).then_inc(act_sem, 1)          # 2
            scalar.wait_ge(sem_b2, 64)
            scalar.activation(cb2[:], tin["beta2"][:], ACTF.Sin, scale=0.5,
                              bias=pi2[:]).then_inc(act_sem, 1)        # 3
            scalar.activation(sb2_[:], tin["beta2"][:], ACTF.Sin,
                              scale=0.5).then_inc(act_sem, 1)          # 4
            scalar.wait_ge(vec_sem, 5)   # all f tiles written
            for n in ("As", "Ac", "Bs", "Bc"):
                scalar.activation(sct[n][:], fts[n][:], ACTF.Sin,
                                  scale=TWOPI).then_inc(act_sem, 1)    # 5..8
            scalar.wait_ge(sem_m, 64)
            scalar.activation(msq[:], tin["m"][:], ACTF.Square).then_inc(act_sem, 1)  # 9
            for r in range(R):
                scalar.activation(wp1[:, _rs(r)], msq[:, _rs(r)], ACTF.Copy,
                                  scale=-cR[r],
                                  bias=float(f32(cR[r]*m0sq[r] + k1[r]))
                                  ).then_inc(act_sem, 1)               # 10..13
            for r in range(R):
                scalar.activation(wp2[:, _rs(r)], msq[:, _rs(r)], ACTF.Copy,
                                  scale=-cI[r],
                                  bias=float(f32(cI[r]*m0sq[r] + k2[r]))
                                  ).then_inc(act_sem, 1)               # 14..17
            for n in ("Cs", "Cc", "Ds", "Dc"):
                scalar.activation(sct[n][:], fts[n][:], ACTF.Sin,
                                  scale=TWOPI).then_inc(act_sem, 1)    # 18..21
            scalar.wait_ge(vec_sem, 6)   # wre
            scalar.activation(wreh[:], wre[:], ACTF.Copy).then_inc(act_sem, 1)  # 22
            scalar.wait_ge(vec_sem, 7)   # wim
            scalar.activation(wimh[:], wim[:], ACTF.Copy).then_inc(act_sem, 1)  # 23
            scalar.wait_ge(vec_sem, 8)   # dg
            scalar.activation(acc[:], dg[:], ACTF.Copy, scale=7.0).then_inc(act_sem, 1)  # 24

        # ------------- VECTOR (DVE) -------------
        # vec_sem: 1 memset, 2-5 f r-groups, 6 wre, 7 wim, 8 dg, 9 final acc
        @block.vector
        def _(vector):
            nc.vector.memset(pi2[:], HALFPI).then_inc(vec_sem, 1)  # 1
            def alpha_bases(r):
                s = _rs(r)
                vector.wait_ge(sem_ab[r], 32)
                nc.vector.tensor_add(ub[:, s], tin["alpha1"][:, s], tin["alpha2"][:, s])
                nc.vector.tensor_sub(vb[:, s], tin["alpha1"][:, s], tin["alpha2"][:, s])

            specs = [("As", ub, wb, RR_ADD, 0.0), ("Ac", ub, wb, RR_ADD, 0.25),
                     ("Bs", vb, zb, RR_SUB, 0.0), ("Bc", vb, zb, RR_SUB, 0.25),
                     ("Cs", ub, zb, RR_ADD, 0.5), ("Cc", ub, zb, RR_ADD, 0.25),
                     ("Ds", vb, wb, RR_SUB, 0.5), ("Dc", vb, wb, RR_SUB, 0.25)]

            def gamma_work(r):
                s = _rs(r)
                vector.wait_ge(sem_gz[r], 32)
                nc.vector.tensor_add(wb[:, s], tin["gamma1"][:, s], tin["gamma2"][:, s])
                nc.vector.tensor_sub(zb[:, s], tin["gamma1"][:, s], tin["gamma2"][:, s])
                for j, (n, xa, xb, op, off) in enumerate(specs):
                    inst = nc.vector._custom_dve(op, out=fts[n][:, s], in0=xa[:, s],
                                                 in1=xb[:, s], s0=INV4PI, s1=off,
                                                 imm2=MAGIC)
                    if j == len(specs) - 1:
                        inst.then_inc(vec_sem, 1)   # r-group done (2..5)

            alpha_bases(0); alpha_bases(1)
            gamma_work(0); gamma_work(1)
            alpha_bases(2); alpha_bases(3)
            gamma_work(2); gamma_work(3)
            # Wigner magnitudes (ACT 1..4)
            vector.wait_ge(act_sem, 4)
            nc.vector.tensor_mul(Ms[0][:], cb1[:], cb2[:])
            nc.vector.tensor_mul(Ms[1][:], sb1[:], sb2_[:])
            nc.vector.tensor_mul(Ms[2][:], cb1[:], sb2_[:])
            nc.vector.tensor_mul(Ms[3][:], sb1[:], cb2[:])
            # pq products for A/B (ACT 5..8)
            for i, (n, M) in enumerate([("As", 0), ("Ac", 0), ("Bs", 1), ("Bc", 1)]):
                vector.wait_ge(act_sem, 5 + i)
                nc.vector.tensor_mul(pqs[n][:], Ms[M][:], sct[n][:])
            nc.vector.tensor_sub(are[:], pqs["Ac"][:], pqs["Bc"][:])
            nc.vector.tensor_sub(aim[:], pqs["Bs"][:], pqs["As"][:])
            # Breit-Wigner (msq = ACT 9, wp1 = ..13, wp2 = ..17)
            vector.wait_ge(act_sem, 9)
            for r in range(R):
                nc.vector._custom_dve(DEN, out=den[:, _rs(r)], in0=msq[:, _rs(r)],
                                      s0=m0sq[r], s1=ysq[r])
            nc.vector.reciprocal_approx_fast(out=rcp[:], in_=den[:])
            vector.wait_ge(act_sem, 13)
            nc.vector.tensor_mul(wre[:], wp1[:], rcp[:]).then_inc(vec_sem, 1)   # 6
            vector.wait_ge(act_sem, 17)
            nc.vector.tensor_mul(wim[:], wp2[:], rcp[:]).then_inc(vec_sem, 1)   # 7
            nc.vector._custom_dve(CHI, out=dall[:], in0=wre[:], in1=wim[:],
                                  s0=1.0, s1=1.0, imm2=0.0)
            nc.vector.tensor_add(dh[:], dall[:, 0:2*E], dall[:, 2*E:4*E])
            nc.vector.tensor_add(dg[:], dh[:, 0:E], dh[:, E:2*E]).then_inc(vec_sem, 1)  # 8
            # pq products for C/D (ACT 18..21)
            for i, (n, M) in enumerate([("Cs", 2), ("Cc", 2), ("Ds", 3), ("Dc", 3)]):
                vector.wait_ge(act_sem, 18 + i)
                nc.vector.tensor_mul(pqs[n][:], Ms[M][:], sct[n][:])
            nc.vector.tensor_add(bre[:], pqs["Cc"][:], pqs["Dc"][:])
            nc.vector.tensor_add(bim[:], pqs["Cs"][:], pqs["Ds"][:])
            # pairs: products/sums per shift group, results written at group
            # offsets into 6E-wide tiles; chi/gw/term merged over all 6 blocks
            vector.wait_ge(act_sem, 23)   # wreh/wimh ready
            ab4v = AB4.rearrange("p (c w) -> p c w", c=4, w=W)
            whv = WH.rearrange("p (c w) -> p c w", c=2, w=W)
            gt6v = GT6.rearrange("p (c w) -> p c w", c=2, w=NP6)
            goffs = {1: 0, 2: 3 * E, 3: 5 * E}
            for sig in (1, 2, 3):
                n = (R - sig) * E
                go = goffs[sig]
                L = slice(0, n)
                Rr = slice(sig * E, sig * E + n)
                # 4 Re-products in one packed op: PT = [aLaR|iLiR|bLbR|jLjR]
                ptv = PT.rearrange("p (c w) -> p c w", c=4, w=NP3)
                nc.vector.tensor_mul(ptv[:, :, :n], ab4v[:, :, L], ab4v[:, :, Rr])
                nc.vector.tensor_add(
                    S1.rearrange("p (c w) -> p c w", c=2, w=NP3)[:, :, :n],
                    ptv[:, 0:2, :n], ptv[:, 2:4, :n])
                nc.vector.tensor_add(avr[:, go:go+n], S1[:, :n], S1[:, NP3:NP3+n])
                # Im-products into QT quarters ordered [q1|q4|q2|q3] so the
                # two differences are one chunked op
                qtv = QT.rearrange("p (c w) -> p c w", c=4, w=NP3)
                nc.vector.tensor_mul(qtv[:, 0, :n], are[:, L], aim[:, Rr])  # q1
                nc.vector.tensor_mul(qtv[:, 1, :n], bim[:, L], bre[:, Rr])  # q4
                nc.vector.tensor_mul(qtv[:, 2, :n], aim[:, L], are[:, Rr])  # q2
                nc.vector.tensor_mul(qtv[:, 3, :n], bre[:, L], bim[:, Rr])  # q3
                nc.vector.tensor_sub(
                    DT.rearrange("p (c w) -> p c w", c=2, w=NP3)[:, :, :n],
                    qtv[:, 0:2, :n], qtv[:, 2:4, :n])
                nc.vector.tensor_add(avi[:, go:go+n], DT[:, :n], DT[:, NP3:NP3+n])
                nc.vector.tensor_mul(gt6v[:, :, go:go+n], whv[:, :, L], whv[:, :, Rr])
            nc.vector._custom_dve(CHI, out=chis[:], in0=avr[:], in1=avi[:],
                                  s0=20.0, s1=4.0, imm2=-6.0)
            nc.vector.tensor_add(gw[:], GT6[:, 0:NP6], GT6[:, NP6:2*NP6])
            nc.vector.tensor_mul(term[:], chis[:], gw[:])
            # tree-sum the 6 pair blocks: one packed 3-pair add, then merge
            tv = term.rearrange("p (c w) -> p c w", c=6, w=E)
            s3 = DT.rearrange("p (c w) -> p c w", c=2, w=NP3)  # reuse DT scratch
            nc.vector.tensor_tensor(s3[:, 0, 0:3*E].rearrange("p (c w) -> p c w", c=3, w=E),
                                    tv[:, 0::2, :], tv[:, 1::2, :], ALU.add)
            nc.vector.tensor_add(DT[:, 3*E:4*E], DT[:, 0:E], DT[:, E:2*E])
            vector.wait_ge(act_sem, 24)   # acc initialised by ACT
            nc.vector.tensor_add(acc[:], acc[:], DT[:, 2*E:3*E])
            nc.vector.tensor_add(acc[:], acc[:], DT[:, 3*E:4*E]) \
                .then_inc(vec_sem, 1)   # 9 -> releases output DMA

    nc.compile()
    return nc


_CACHE = {}


def kernel(alpha1, beta1, gamma1, alpha2, beta2, gamma2, m, m0, g0,
           coef_r, coef_i, _want_trace=False):
    key = (np.asarray(m0, np.float32).tobytes(), np.asarray(g0, np.float32).tobytes(),
           np.asarray(coef_r, np.float32).tobytes(), np.asarray(coef_i, np.float32).tobytes())
    if key not in _CACHE:
        _CACHE[key] = build(np.asarray(m0, np.float32), np.asarray(g0, np.float32),
                            np.asarray(coef_r, np.float32), np.asarray(coef_i, np.float32))
    nc = _CACHE[key]
    full = {"alpha1": alpha1, "beta1": beta1, "gamma1": gamma1,
            "alpha2": alpha2, "beta2": beta2, "gamma2": gamma2, "m": m}
    in_maps = []
    for i in range(N_CORES):
        sl = slice(i * N_CORE, (i + 1) * N_CORE)
        in_maps.append({k: np.ascontiguousarray(np.asarray(v, np.float32)[:, sl])
                        for k, v in full.items()})
    res = run_bass_kernel_spmd(nc, in_maps, core_ids=list(range(N_CORES)),
                               trace=_want_trace)
    out = np.concatenate([res.results[i]["out"] for i in range(N_CORES)])
    if _want_trace:
        kernel._last_result = res
    return out.astype(np.float32)



# revision 5
# speedup vs baseline: 1.0353x; 1.0353x over previous
"""Trainium2 Bass kernel for nn_AllAmplitude (helicity-amplitude intensity).

Math: the reference contracts two spin-1 Wigner-D matrices per (resonance,
event) with a Breit-Wigner weight and sums |amp|^2 over external helicities.
Because D1 @ D2 = D^1(U1 U2) for the SU(2) elements U1, U2 of the two
rotations, the whole intensity collapses to

  I = 7 sum_r |w_r|^2
    + sum_{r<r'} 2 Re(w_r conj(w_r')) (10 Re(av)^2 + 2 Im(av)^2 - 3)

with av = conj(a_r') a_r + b_r' conj(b_r), (a, b) the Cayley-Klein parameters
of the composed rotation, and w_r the complex Breit-Wigner weight.

v2 changes vs the original kernel:
 - 4 fused range-reduction ops (fp16 out) instead of 8: the cosine variants
   come from cos(2*pi*f) = sin(pi/2 - 2*pi*|f|), with |f| produced by a
   single packed 4x-mode int16 AND-mask tensor_scalar; the C/D sign flips
   fold into the ACT sin scale (-2*pi).
 - |w_r|^2 = |coef_r|^2 / den_r: the diagonal term is an exact 2x-mode
   tensor_scalar on the reciprocal (replaces a 1x custom square op).
 - DMA issue split across Sync (alpha1, alpha2, beta1, beta2, m) and the
   Scalar engine (gamma1, gamma2) so the 28 input slices reach the 16 HWDGE
   queues ~2x sooner (Sync-issued DMAs cost ~565ns of issue time each).
 - half-tile A-stage (range reduction r01/r23) + 2-channel packed ACT sins
   for earlier pipeline starts; split output DMA.

Sharding: pure data parallelism over the event axis N=262144 across the 8
NeuronCores (32768 events each, laid out [128 partitions x 256 events] with
the R=4 resonance slices side by side in the free axis).
"""

import numpy as np

import concourse.bass as bass
from concourse import bacc, mybir
from concourse.bass_utils import run_bass_kernel_spmd

F32 = mybir.dt.float32
BF16 = mybir.dt.bfloat16
FP16 = mybir.dt.float16
I16 = mybir.dt.int16
ALU = mybir.AluOpType
ACTF = mybir.ActivationFunctionType

R = 4
N_TOTAL = 262144
N_CORES = 8
N_CORE = N_TOTAL // N_CORES     # 32768 events per core
P = 128                         # SBUF partitions
E = N_CORE // P                 # 256 events per partition per resonance
W = R * E                       # 1024 free-dim of a full working tile
H = W // 2                      # half tile (r01 / r23)

MAGIC = float(np.float32(1.5 * 2.0**23))   # round-to-nearest-int bias trick
INV4PI = float(np.float32(1.0 / (4.0 * np.pi)))
TWOPI = float(np.float32(2.0 * np.pi))
HALFPI = float(np.float32(np.pi / 2.0))

INPUT_NAMES = ("alpha1", "beta1", "gamma1", "alpha2", "beta2", "gamma2", "m")


def _register_custom_ops():
    import concourse.dve_ops as dve_ops
    from concourse.dve_spec import Spec, Src0, Src1, C0, C1, C2, sq, lower, _has_src1
    from concourse.dve_uop import DveOpSpec
    from concourse.dve_ops import DveOp

    if any(op.name == "ANT_RANGE_RED_ADD" for op in dve_ops.OPS):
        return {op.name: op for op in dve_ops.OPS}

    def make_op(name, spec):
        shas = {}
        for ver in ("v3", "v4"):
            uops = lower(spec, ver=ver)
            shas[ver] = DveOpSpec(name=name, opcode=31, uops=uops,
                                  rd1_en=_has_src1(spec)).sha(ver)
        return DveOp(name, spec, subdim=False, uops_sha=shas)

    def _rr_ref(sgn):
        def ref(in0, in1, s0, s1, imm2):
            t = ((in0 + sgn * in1) * s0 + s1).astype(np.float32)
            r = ((t + imm2).astype(np.float32) - imm2).astype(np.float32)
            return (t - r).astype(np.float32)
        return ref

    u = (Src0 + Src1) * C0 + C1
    rr_add = make_op("ANT_RANGE_RED_ADD",
                     Spec(body=u - ((u + C2) - C2), reference=_rr_ref(1.0)))
    u2 = (Src0 - Src1) * C0 + C1
    rr_sub = make_op("ANT_RANGE_RED_SUB",
                     Spec(body=u2 - ((u2 + C2) - C2), reference=_rr_ref(-1.0)))
    chi = make_op("ANT_CHI", Spec(
        body=sq(Src0) * C0 + sq(Src1) * C1 + C2,
        reference=lambda in0, in1, s0, s1, imm2:
            (in0 * in0 * s0 + in1 * in1 * s1 + imm2).astype(np.float32)))
    den = make_op("ANT_DEN", Spec(
        body=sq(C0 - Src0) + C1,
        reference=lambda in0, in1, s0, s1, imm2:
            ((s0 - in0) * (s0 - in0) + s1).astype(np.float32)))

    for op in (rr_add, rr_sub, chi, den):
        dve_ops.OPS.append(op)
        dve_ops._SUB_OPCODE_FOR_NAME[op.name] = (
            dve_ops._CUSTOM_DVE_ROW_BASE + len(dve_ops.OPS) - 1)
        dve_ops.CUSTOM_DVE_SPECS[op.name] = op.spec
    assert max(dve_ops._SUB_OPCODE_FOR_NAME.values()) < 0x20
    return {op.name: op for op in dve_ops.OPS}


def _rs(r):
    return slice(r * E, (r + 1) * E)


def build(m0, g0, coef_r, coef_i):
    OPS = _register_custom_ops()
    RR_ADD, RR_SUB, CHI, DEN = (OPS["ANT_RANGE_RED_ADD"], OPS["ANT_RANGE_RED_SUB"],
                                OPS["ANT_CHI"], OPS["ANT_DEN"])
    AT = FP16

    nc = bacc.Bacc("TRN2", target_bir_lowering=False, debug=False,
                   num_devices=N_CORES)
    ins = {k: nc.dram_tensor(k, (R, N_CORE), F32, kind="ExternalInput").ap()
           for k in INPUT_NAMES}
    out_ap = nc.dram_tensor("out", (N_CORE,), F32, kind="ExternalOutput").ap()

    f32 = np.float32
    m0 = m0.astype(np.float64); g0 = g0.astype(np.float64)
    cR = [float(f32(coef_r[r] * np.cos(coef_i[r]))) for r in range(R)]
    cI = [float(f32(coef_r[r] * np.sin(coef_i[r]))) for r in range(R)]
    m0sq = [float(f32(m0[r] * m0[r])) for r in range(R)]
    y = [float(f32(m0[r] * g0[r])) for r in range(R)]
    ysq = [float(f32(f32(y[r]) * f32(y[r]))) for r in range(R)]
    k1 = [float(f32(-f32(cI[r]) * f32(y[r]))) for r in range(R)]
    k2 = [float(f32(f32(cR[r]) * f32(y[r]))) for r in range(R)]
    c27 = [float(f32(7.0 * (f32(cR[r])**2 + f32(cI[r])**2))) for r in range(R)]

    # ---- static SBUF allocation ----
    alloc = []
    def sb(name, shape, dt=F32):
        t = nc.alloc_sbuf_tensor(name, list(shape), dt)
        alloc.append(t)
        return t.ap()

    tin = {k: sb(f"in_{k}", [P, W]) for k in INPUT_NAMES}
    pi2 = sb("pi2", [P, 1])
    ub = sb("ub", [P, W]); vb = sb("vb", [P, W])
    wb = sb("wb", [P, W]); zb = sb("zb", [P, W])
    # packed fracs [fA|fB|fC|fD] and their absolute values, fp16
    f4 = sb("f4", [P, 4 * W], AT)
    a4 = sb("a4", [P, 4 * W], AT)
    fA = f4[:, 0:W]; fB = f4[:, W:2*W]; fC = f4[:, 2*W:3*W]; fD = f4[:, 3*W:4*W]
    aA = a4[:, 0:W]; aB = a4[:, W:2*W]; aC = a4[:, 2*W:3*W]; aD = a4[:, 3*W:4*W]
    # sct tiles (sin/cos of composite angles), packed [As|Bs|Cs|Ds] / [Ac|..]
    sct_s4 = sb("sct_s4", [P, 4 * W], AT)
    sct_c4 = sb("sct_c4", [P, 4 * W], AT)
    sct = {"As": sct_s4[:, 0:W], "Bs": sct_s4[:, W:2*W],
           "Cs": sct_s4[:, 2*W:3*W], "Ds": sct_s4[:, 3*W:4*W],
           "Ac": sct_c4[:, 0:W], "Bc": sct_c4[:, W:2*W],
           "Cc": sct_c4[:, 2*W:3*W], "Dc": sct_c4[:, 3*W:4*W]}
    cb1 = sb("cb1", [P, W], AT); sb1 = sb("sb1", [P, W], AT)
    cb2 = sb("cb2", [P, W], AT); sb2_ = sb("sb2", [P, W], AT)
    Ms = [sb(f"M{i}", [P, W], AT) for i in range(4)]
    pqs = {n: sb(f"pq_{n}", [P, W], AT) for n in sct}
    AB4 = sb("AB4", [P, 4 * W], AT)   # [are|aim|bre|bim]
    are = AB4[:, 0:W]; aim = AB4[:, W:2*W]
    bre = AB4[:, 2*W:3*W]; bim = AB4[:, 3*W:4*W]
    msq = sb("msq", [P, W]); den = sb("den", [P, W]); rcp = den
    wp1 = sb("wp1", [P, W]); wp2 = sb("wp2", [P, W])
    wre = sb("wre", [P, W]); wim = sb("wim", [P, W])
    WH = sb("WH", [P, 2 * W], BF16)   # [wreh|wimh]
    wreh = WH[:, 0:W]; wimh = WH[:, W:2*W]
    dall = sb("dall", [P, W]); dh = sb("dh", [P, 2 * E]); dg = sb("dg", [P, E])
    acc = sb("acc", [P, E])
    # pair scratch (reused across the 3 shift groups; DVE program order)
    NP3 = 3 * E
    PT = sb("PT", [P, 4 * NP3], AT)    # 4 packed products
    S1 = sb("S1", [P, 2 * NP3], AT)
    QT = sb("QT", [P, 4 * NP3], AT)    # [q1|q4|q2|q3]
    DT = sb("DT", [P, 2 * NP3], AT)
    NP6 = 6 * E
    avr = sb("avr", [P, NP6], AT); avi = sb("avi", [P, NP6], AT)
    chis = sb("chis", [P, NP6], BF16); gw = sb("gw", [P, NP6], BF16)
    GT6 = sb("GT6", [P, 2 * NP6], BF16)
    term = sb("term", [P, NP6], BF16)

    sem_aa = [nc.alloc_semaphore(f"s_aa{r}") for r in range(R)]  # alpha1+alpha2 per r
    sem_gg = [nc.alloc_semaphore(f"s_gg{r}") for r in range(R)]  # gamma1+gamma2 per r
    with (
        nc.semaphore("s_b1") as sem_b1,
        nc.semaphore("s_b2") as sem_b2,
        nc.semaphore("s_m") as sem_m,
        nc.semaphore("s_out") as sem_out,
        nc.semaphore("act_sem") as act_sem,
        nc.semaphore("vec_sem") as vec_sem,
        nc.Block() as block,
    ):
        def dma(eng, k, r, sem):
            eng.dma_start(
                tin[k][:, _rs(r)],
                ins[k][r].rearrange("(p e) -> p e", p=P, e=E),
            ).then_inc(sem, 16)

        # ------------- SYNC: alpha1, alpha2, beta1, beta2, m -------------
        @block.sync
        def _(sync):
            for r in range(R):
                dma(sync, "alpha1", r, sem_aa[r])
            for r in range(R):
                dma(sync, "alpha2", r, sem_aa[r])
            for r in range(R):
                dma(sync, "beta1", r, sem_b1)
            for r in range(R):
                dma(sync, "beta2", r, sem_b2)
            for r in range(R):
                dma(sync, "m", r, sem_m)
            # split output: two column-halves of the (p, e) view, 2 queues
            outv = out_ap.rearrange("(p e) -> p e", p=P, e=E)
            sync.wait_ge(vec_sem, 16)
            sync.dma_start(outv[:, 0:E//2], acc[:, 0:E//2]).then_inc(sem_out, 16)
            sync.wait_ge(vec_sem, 17)
            sync.dma_start(outv[:, E//2:], acc[:, E//2:]).then_inc(sem_out, 16)
            sync.wait_ge(sem_out, 32)

        # ------------- SCALAR (ACT): gamma DMAs, then all transcendentals ----
        # act_sem: 1 cb1, 2 sb1, 3 cb2, 4 sb2,
        #          5 AsBs01, 6 CsDs01, 7 AcBc01, 8 CcDc01,
        #          9 AsBs23, 10 CsDs23, 11 AcBc23, 12 CcDc23,
        #          13 msq, 14-17 wp1, 18-21 wp2, 22 wreh, 23 wimh
        @block.scalar
        def _(scalar):
            for r in range(R):
                dma(scalar, "gamma1", r, sem_gg[r])
            for r in range(R):
                dma(scalar, "gamma2", r, sem_gg[r])
            scalar.wait_ge(vec_sem, 1)   # pi2 memset
            # beta sins (beta1/beta2 land early; issued 3rd/4th on sync)
            scalar.wait_ge(sem_b1, 64)
            scalar.activation(cb1[:], tin["beta1"][:], ACTF.Sin, scale=0.5,
                              bias=pi2[:]).then_inc(act_sem, 1)        # 1
            scalar.activation(sb1[:], tin["beta1"][:], ACTF.Sin,
                              scale=0.5).then_inc(act_sem, 1)          # 2
            scalar.wait_ge(sem_b2, 64)
            scalar.activation(cb2[:], tin["beta2"][:], ACTF.Sin, scale=0.5,
                              bias=pi2[:]).then_inc(act_sem, 1)        # 3
            scalar.activation(sb2_[:], tin["beta2"][:], ACTF.Sin,
                              scale=0.5).then_inc(act_sem, 1)          # 4

            s4v = sct_s4.rearrange("p (c w) -> p c w", c=4, w=W)
            c4v = sct_c4.rearrange("p (c w) -> p c w", c=4, w=W)
            f4v = f4.rearrange("p (c w) -> p c w", c=4, w=W)
            a4v = a4.rearrange("p (c w) -> p c w", c=4, w=W)
            for h in range(2):
                s = slice(h * H, h * H + H)
                scalar.wait_ge(vec_sem, 3 + 6 * h)    # fA,fB half h
                scalar.activation(s4v[:, 0:2, s], f4v[:, 0:2, s], ACTF.Sin,
                                  scale=TWOPI).then_inc(act_sem, 1)   # 5/9
                scalar.wait_ge(vec_sem, 5 + 6 * h)    # fC,fD half h
                scalar.activation(s4v[:, 2:4, s], f4v[:, 2:4, s], ACTF.Sin,
                                  scale=-TWOPI).then_inc(act_sem, 1)  # 6/10
                scalar.wait_ge(vec_sem, 7 + 6 * h)    # abs half h (all four)
                scalar.activation(c4v[:, 0:2, s], a4v[:, 0:2, s], ACTF.Sin,
                                  scale=-TWOPI, bias=pi2[:]).then_inc(act_sem, 1)  # 7/11
                scalar.activation(c4v[:, 2:4, s], a4v[:, 2:4, s], ACTF.Sin,
                                  scale=-TWOPI, bias=pi2[:]).then_inc(act_sem, 1)  # 8/12

            scalar.wait_ge(sem_m, 64)
            scalar.activation(msq[:], tin["m"][:], ACTF.Square).then_inc(act_sem, 1)  # 13
            for r in range(R):
                scalar.activation(wp1[:, _rs(r)], msq[:, _rs(r)], ACTF.Copy,
                                  scale=-cR[r],
                                  bias=float(f32(cR[r]*m0sq[r] + k1[r]))
                                  ).then_inc(act_sem, 1)               # 14..17
            for r in range(R):
                scalar.activation(wp2[:, _rs(r)], msq[:, _rs(r)], ACTF.Copy,
                                  scale=-cI[r],
                                  bias=float(f32(cI[r]*m0sq[r] + k2[r]))
                                  ).then_inc(act_sem, 1)               # 18..21
            scalar.wait_ge(vec_sem, 14)   # wre
            scalar.activation(wreh[:], wre[:], ACTF.Copy).then_inc(act_sem, 1)  # 22
            scalar.wait_ge(vec_sem, 15)   # wim
            scalar.activation(wimh[:], wim[:], ACTF.Copy).then_inc(act_sem, 1)  # 23

        # ------------- VECTOR (DVE) -------------
        # vec_sem: 1 memset; per half h: 3+6h fAB, 5+6h fCD, 7+6h abs
        #   (h=0: 2..3 used as (2=fA.. we inc after fB), see below)
        # 14 wre, 15 wim, 16/17 acc halves
        @block.vector
        def _(vector):
            nc.vector.memset(pi2[:], HALFPI).then_inc(vec_sem, 1)  # 1

            # ---- stage A: quarter pre-adds + half-tile RR + packed abs ----
            def pre_r(r):
                s = _rs(r)
                vector.wait_ge(sem_aa[r], 32)
                nc.vector.tensor_add(ub[:, s], tin["alpha1"][:, s], tin["alpha2"][:, s])
                nc.vector.tensor_sub(vb[:, s], tin["alpha1"][:, s], tin["alpha2"][:, s])
                vector.wait_ge(sem_gg[r], 32)
                nc.vector.tensor_add(wb[:, s], tin["gamma1"][:, s], tin["gamma2"][:, s])
                nc.vector.tensor_sub(zb[:, s], tin["gamma1"][:, s], tin["gamma2"][:, s])

            for h in range(2):
                pre_r(2 * h); pre_r(2 * h + 1)
                s = slice(h * H, h * H + H)
                nc.vector._custom_dve(RR_ADD, out=fA[:, s], in0=ub[:, s],
                                      in1=wb[:, s], s0=INV4PI, s1=0.0, imm2=MAGIC)
                nc.vector._custom_dve(RR_SUB, out=fB[:, s], in0=vb[:, s],
                                      in1=zb[:, s], s0=INV4PI, s1=0.0,
                                      imm2=MAGIC).then_inc(vec_sem, 2)   # 3/9
                nc.vector._custom_dve(RR_ADD, out=fC[:, s], in0=ub[:, s],
                                      in1=zb[:, s], s0=INV4PI, s1=0.0, imm2=MAGIC)
                nc.vector._custom_dve(RR_SUB, out=fD[:, s], in0=vb[:, s],
                                      in1=wb[:, s], s0=INV4PI, s1=0.0,
                                      imm2=MAGIC).then_inc(vec_sem, 2)   # 5/11
                # packed |f| for all four combos of this half: 4-ch AP view
                f4i = f4.bitcast(I16).rearrange("p (c w) -> p c w", c=4, w=W)
                a4i = a4.bitcast(I16).rearrange("p (c w) -> p c w", c=4, w=W)
                nc.vector.tensor_scalar(a4i[:, :, s], f4i[:, :, s], 0x7FFF, None,
                                        ALU.bitwise_and).then_inc(vec_sem, 2)  # 7/13

            # ---- stage B: Wigner magnitudes + pq products ----
            vector.wait_ge(act_sem, 4)
            nc.vector.tensor_mul(Ms[0][:], cb1[:], cb2[:])
            nc.vector.tensor_mul(Ms[1][:], sb1[:], sb2_[:])
            nc.vector.tensor_mul(Ms[2][:], cb1[:], sb2_[:])
            nc.vector.tensor_mul(Ms[3][:], sb1[:], cb2[:])
            # pq for A/B (needs act 5..12 done up to the right tiles)
            vector.wait_ge(act_sem, 9)   # AsBs both halves
            nc.vector.tensor_mul(pqs["As"][:], Ms[0][:], sct["As"][:])
            nc.vector.tensor_mul(pqs["Bs"][:], Ms[1][:], sct["Bs"][:])
            vector.wait_ge(act_sem, 11)  # AcBc both halves
            nc.vector.tensor_mul(pqs["Ac"][:], Ms[0][:], sct["Ac"][:])
            nc.vector.tensor_mul(pqs["Bc"][:], Ms[1][:], sct["Bc"][:])
            nc.vector.tensor_sub(are[:], pqs["Ac"][:], pqs["Bc"][:])
            nc.vector.tensor_sub(aim[:], pqs["Bs"][:], pqs["As"][:])
            vector.wait_ge(act_sem, 10)  # CsDs both halves
            nc.vector.tensor_mul(pqs["Cs"][:], Ms[2][:], sct["Cs"][:])
            nc.vector.tensor_mul(pqs["Ds"][:], Ms[3][:], sct["Ds"][:])
            vector.wait_ge(act_sem, 12)  # CcDc both halves
            nc.vector.tensor_mul(pqs["Cc"][:], Ms[2][:], sct["Cc"][:])
            nc.vector.tensor_mul(pqs["Dc"][:], Ms[3][:], sct["Dc"][:])
            nc.vector.tensor_add(bre[:], pqs["Cc"][:], pqs["Dc"][:])
            nc.vector.tensor_add(bim[:], pqs["Cs"][:], pqs["Ds"][:])

            # ---- stage C: Breit-Wigner ----
            vector.wait_ge(act_sem, 13)   # msq
            for r in range(R):
                nc.vector._custom_dve(DEN, out=den[:, _rs(r)], in0=msq[:, _rs(r)],
                                      s0=m0sq[r], s1=ysq[r])
            nc.vector.reciprocal_approx_fast(out=rcp[:], in_=den[:])
            vector.wait_ge(act_sem, 17)
            nc.vector.tensor_mul(wre[:], wp1[:], rcp[:]).then_inc(vec_sem, 1)   # 14
            vector.wait_ge(act_sem, 21)
            nc.vector.tensor_mul(wim[:], wp2[:], rcp[:]).then_inc(vec_sem, 1)   # 15
            # diagonal: 7*|w_r|^2 = 7*|coef_r|^2 / den_r  (exact, 2x-mode TS)
            for r in range(R):
                nc.vector.tensor_scalar(dall[:, _rs(r)], rcp[:, _rs(r)],
                                        c27[r], None, ALU.mult)
            nc.vector.tensor_add(dh[:], dall[:, 0:2*E], dall[:, 2*E:4*E])
            nc.vector.tensor_add(dg[:], dh[:, 0:E], dh[:, E:2*E])

            # ---- stage D: pairs ----
            vector.wait_ge(act_sem, 23)   # wreh/wimh ready
            ab4v = AB4.rearrange("p (c w) -> p c w", c=4, w=W)
            whv = WH.rearrange("p (c w) -> p c w", c=2, w=W)
            gt6v = GT6.rearrange("p (c w) -> p c w", c=2, w=NP6)
            goffs = {1: 0, 2: 3 * E, 3: 5 * E}
            for sig in (1, 2, 3):
                n = (R - sig) * E
                go = goffs[sig]
                L = slice(0, n)
                Rr = slice(sig * E, sig * E + n)
                ptv = PT.rearrange("p (c w) -> p c w", c=4, w=NP3)
                nc.vector.tensor_mul(ptv[:, :, :n], ab4v[:, :, L], ab4v[:, :, Rr])
                nc.vector.tensor_add(
                    S1.rearrange("p (c w) -> p c w", c=2, w=NP3)[:, :, :n],
                    ptv[:, 0:2, :n], ptv[:, 2:4, :n])
                nc.vector.tensor_add(avr[:, go:go+n], S1[:, :n], S1[:, NP3:NP3+n])
                qtv = QT.rearrange("p (c w) -> p c w", c=4, w=NP3)
                nc.vector.tensor_mul(qtv[:, 0, :n], are[:, L], aim[:, Rr])  # q1
                nc.vector.tensor_mul(qtv[:, 1, :n], bim[:, L], bre[:, Rr])  # q4
                nc.vector.tensor_mul(qtv[:, 2, :n], aim[:, L], are[:, Rr])  # q2
                nc.vector.tensor_mul(qtv[:, 3, :n], bre[:, L], bim[:, Rr])  # q3
                nc.vector.tensor_sub(
                    DT.rearrange("p (c w) -> p c w", c=2, w=NP3)[:, :, :n],
                    qtv[:, 0:2, :n], qtv[:, 2:4, :n])
                nc.vector.tensor_add(avi[:, go:go+n], DT[:, :n], DT[:, NP3:NP3+n])
                nc.vector.tensor_mul(gt6v[:, :, go:go+n], whv[:, :, L], whv[:, :, Rr])
            nc.vector._custom_dve(CHI, out=chis[:], in0=avr[:], in1=avi[:],
                                  s0=20.0, s1=4.0, imm2=-6.0)
            nc.vector.tensor_add(gw[:], GT6[:, 0:NP6], GT6[:, NP6:2*NP6])
            nc.vector.tensor_mul(term[:], chis[:], gw[:])
            # tree-sum the 6 pair blocks: one packed 3-pair add, then merge
            tv = term.rearrange("p (c w) -> p c w", c=6, w=E)
            s3 = DT.rearrange("p (c w) -> p c w", c=2, w=NP3)  # reuse DT scratch
            nc.vector.tensor_tensor(s3[:, 0, 0:3*E].rearrange("p (c w) -> p c w", c=3, w=E),
                                    tv[:, 0::2, :], tv[:, 1::2, :], ALU.add)
            nc.vector.tensor_add(DT[:, 3*E:4*E], DT[:, 0:E], DT[:, E:2*E])
            nc.vector.tensor_add(dh[:, 0:E], DT[:, 2*E:3*E], DT[:, 3*E:4*E])
            # final: acc = pairs + diagonal, split in halves for output DMA
            nc.vector.tensor_add(acc[:, 0:E//2], dh[:, 0:E//2], dg[:, 0:E//2]) \
                .then_inc(vec_sem, 1)   # 16
            nc.vector.tensor_add(acc[:, E//2:E], dh[:, E//2:E], dg[:, E//2:E]) \
                .then_inc(vec_sem, 1)   # 17

    nc.compile()
    return nc


_CACHE = {}


def kernel(alpha1, beta1, gamma1, alpha2, beta2, gamma2, m, m0, g0,
           coef_r, coef_i, _want_trace=False):
    key = (np.asarray(m0, np.float32).tobytes(), np.asarray(g0, np.float32).tobytes(),
           np.asarray(coef_r, np.float32).tobytes(), np.asarray(coef_i, np.float32).tobytes())
    if key not in _CACHE:
        _CACHE[key] = build(np.asarray(m0, np.float32), np.asarray(g0, np.float32),
                            np.asarray(coef_r, np.float32), np.asarray(coef_i, np.float32))
    nc = _CACHE[key]
    full = {"alpha1": alpha1, "beta1": beta1, "gamma1": gamma1,
            "alpha2": alpha2, "beta2": beta2, "gamma2": gamma2, "m": m}
    in_maps = []
    for i in range(N_CORES):
        sl = slice(i * N_CORE, (i + 1) * N_CORE)
        in_maps.append({k: np.ascontiguousarray(np.asarray(v, np.float32)[:, sl])
                        for k, v in full.items()})
    res = run_bass_kernel_spmd(nc, in_maps, core_ids=list(range(N_CORES)),
                               trace=_want_trace)
    out = np.concatenate([res.results[i]["out"] for i in range(N_CORES)])
    if _want_trace:
        kernel._last_result = res
    return out.astype(np.float32)
